# revision 27
# baseline (speedup 1.0000x reference)
"""Trainium2 Bass kernel for bag-level attention (ragged_sequence).

Math (per bag b over its 16 sentences i):
    att_i  = <x_i, rel[q_i]>
    w      = softmax(att) within bag
    logits = (sum_i w_i x_i) @ rel.T + bias

Key identity: logits[b] = sum_i w_i S[i,:] + bias with S = x @ rel.T, so x is
read from HBM exactly once.

Precision: x and rel are split on the host into fp16 hi + fp16 lo
(x = hi + lo, 22-bit combined mantissa). The four partial products
hi*hi + hi*lo + lo*hi + lo*lo are accumulated in fp32 PSUM, reproducing fp32
accuracy (~1e-6 rel) while running the TensorE at full fp16 rate (fp32
matmuls run at quarter rate and do not warm the HAM clock gate).

Device layout (per core, rows = N/8 sentences):
    S.T split over two partition blocks of PSUM st[128, ch]:
      rows 0:64   = relT_hi(64-col zero-padded).T @ xT_{hi,lo}   (tile_position (0,0))
      rows 64:128 = relT_lo(padded).T @ xT_{hi,lo}               (tile_position (0,64))
    The two col-tiles share each moving stream (concurrent sub-array execution).
    att  = partition_all_reduce(st * onehot2)        (GpSimd; onehot2 has the
           one-hot replicated in both blocks, built on host)
    e    = exp(att)                                  (ScalarE)
    ebs  = partition_broadcast(e)                    (GpSimd)
    lu[128, bags] = windowed reduce_16(st * ebs)     (VectorE)
    logitsU.T[53, bags] = stacked_identity.T @ lu    (recombines hi+lo blocks)
    * 1/z, + bias, final PE transpose to [bags, 53].
"""

import os
from contextlib import ExitStack

import numpy as np

import concourse.bass as bass
import concourse.tile as tile
from concourse import bacc, library_config, mybir
from concourse.bass_utils import run_bass_kernel_spmd

# Problem constants (hardcoded per spec nn_Attention_85478439125349)
N = 262144
B = 16384
D = 768
C = 53
BAG = 16
N_CORES = 8
ROWS = N // N_CORES          # 32768 sentences per core
BAGS = B // N_CORES          # 2048 bags per core
KCH = D // 128               # 6 contraction chunks
F32 = mybir.dt.float32
F16 = mybir.dt.float16


def build_nc(rows: int, sc: int = 2048, ch: int = 512) -> bass.Bass:
    """Build the per-core Bass program for `rows` sentences (bags of BAG)."""
    assert rows % sc == 0 and sc % ch == 0 and ch % BAG == 0
    bags = rows // BAG
    n_sc = rows // sc          # superchunks (DMA granularity)
    n_ch = sc // ch            # compute chunks per superchunk
    chb = ch // BAG            # bags per compute chunk (32)
    scb = sc // BAG            # bags per superchunk (128)

    nc = bacc.Bacc()
    # hi/lo interleaved: xt2[d, 0, r] = fp16 hi, xt2[d, 1, r] = fp16 lo
    xt2 = nc.declare_dram_parameter("xt2", [D, 2, rows], F16, isOutput=False)
    # one-hot mask replicated into both partition blocks: [128, rows]
    oht = nc.declare_dram_parameter("oht", [128, rows], F16, isOutput=False)
    # relT hi/lo, each zero-padded to 64 output columns: [D, 2, 64]
    relt2 = nc.declare_dram_parameter("relt2", [D, 2, 64], F16, isOutput=False)
    # stacked identity [128, C]: row k -> col m if k==m or k==64+m
    sident = nc.declare_dram_parameter("sident", [128, C], F32, isOutput=False)
    identm = nc.declare_dram_parameter("identm", [C, C], F32, isOutput=False)
    biast = nc.declare_dram_parameter("biast", [C, 1], F32, isOutput=False)
    out = nc.declare_dram_parameter("out", [bags, C], F32, isOutput=True)

    xt_v = xt2.rearrange("(k p) h r -> k p h r", p=128)      # [KCH, 128, 2, rows]
    relt_v = relt2.rearrange("(k p) h c -> k p h c", p=128)  # [KCH, 128, 2, 64]

    with tile.TileContext(nc) as tc, ExitStack() as ctx:
        consts = ctx.enter_context(tc.tile_pool(name="consts", bufs=1))
        xpool = ctx.enter_context(tc.tile_pool(name="xpool", bufs=2))
        ohpool = ctx.enter_context(tc.tile_pool(name="ohpool", bufs=2))
        work = ctx.enter_context(tc.tile_pool(name="work", bufs=2))
        psum = ctx.enter_context(tc.tile_pool(name="psum", bufs=2, space="PSUM"))

        # --- constants ---
        relt_sb = consts.tile([128, KCH, 2, 64], F16)
        nc.sync.dma_start(out=relt_sb, in_=relt_v.transpose([1, 0, 2, 3]))
        sident_sb = consts.tile([128, C], F32)
        nc.sync.dma_start(out=sident_sb, in_=sident[:, :])
        bias_sb = consts.tile([C, 1], F32)
        nc.sync.dma_start(out=bias_sb, in_=biast[:, :])
        ident = consts.tile([C, C], F32)
        nc.sync.dma_start(out=ident, in_=identm[:, :])
        zeros_sb = consts.tile([64, 512], F32)
        nc.vector.memset(zeros_sb, 0.0)
        nc.gpsimd.load_library(library_config.attn)
        # accumulator for logits^T [C, bags] and staging for transposed output
        lt_acc = consts.tile([C, bags], F32)
        logits_sb = consts.tile([128, bags // 128, C], F32)

        for isc in range(n_sc):
            x_sb = xpool.tile([128, KCH, 2, sc], F16)
            for h in range(2):
                nc.sync.dma_start(
                    out=x_sb[:, :, h, :],
                    in_=xt_v[:, :, h, isc * sc : (isc + 1) * sc].transpose([1, 0, 2]),
                )
            oh_sb = ohpool.tile([128, sc], F16)
            nc.sync.dma_start(out=oh_sb, in_=oht[:, isc * sc : (isc + 1) * sc])

            for ic in range(n_ch):
                cs = slice(ic * ch, (ic + 1) * ch)
                st = psum.tile([128, ch], F32, tag="st")
                # Zero the lo block: its matmuls use start=False (a second
                # bank-wide has_written clear would wipe the hi block), so on
                # sim/stale PSUM the first accumulate needs zeroed ground.
                nc.scalar.copy(st[64:128, :], zeros_sb[:, :ch])
                for k in range(KCH):
                    for h in range(2):  # moving stream: x hi, then x lo
                        nc.tensor.matmul(
                            st[0:64, :],
                            lhsT=relt_sb[:, k, 0, :],
                            rhs=x_sb[:, k, h, cs],
                            start=(k == 0 and h == 0),
                            stop=(k == KCH - 1 and h == 1),
                            tile_position=(0, 0),
                        )
                        nc.tensor.matmul(
                            st[64:128, :],
                            lhsT=relt_sb[:, k, 1, :],
                            rhs=x_sb[:, k, h, cs],
                            start=False,
                            stop=False,
                            skip_group_check=True,
                            tile_position=(0, 64),
                        )
                # att extraction: sm = st * onehot2 ; att = column sums
                sm = work.tile([128, ch], F32, tag="sm")
                nc.vector.tensor_mul(sm, st, oh_sb[:, cs])
                att = work.tile([128, ch], F32, tag="att")
                nc.gpsimd.partition_all_reduce(
                    att, sm, channels=128, reduce_op=bass.bass_isa.ReduceOp.add
                )
                e = work.tile([1, ch], F32, tag="e")
                nc.scalar.activation(
                    e, att[0:1, :], mybir.ActivationFunctionType.Exp
                )
                # broadcast e across partitions, weight S^T, bag-reduce
                ebs = work.tile([128, ch], F32, tag="ebs")
                nc.gpsimd.partition_broadcast(ebs, e, channels=128)
                w = work.tile([128, ch], F32, tag="w")
                nc.vector.tensor_mul(w, st, ebs)
                lu = work.tile([128, chb], F32, tag="lu")
                nc.vector.reduce_sum(
                    lu, w.rearrange("p (b j) -> p b j", j=BAG), axis=mybir.AxisListType.X
                )
                # recombine hi+lo partition blocks: [53, chb]
                lc = psum.tile([C, chb], F32, tag="lc")
                nc.tensor.matmul(lc, lhsT=sident_sb, rhs=lu)
                # z per bag from the broadcast copy; normalize
                zb = work.tile([C, chb], F32, tag="zb")
                nc.vector.reduce_sum(
                    zb,
                    ebs[0:C, :].rearrange("p (b j) -> p b j", j=BAG),
                    axis=mybir.AxisListType.X,
                )
                rzb = work.tile([C, chb], F32, tag="rzb")
                nc.vector.reciprocal(rzb, zb)
                ob = isc * scb + ic * chb
                nc.vector.tensor_mul(lt_acc[:, ob : ob + chb], lc, rzb)
            # bias for this superchunk's bag block
            bs = slice(isc * scb, (isc + 1) * scb)
            nc.vector.tensor_scalar_add(
                out=lt_acc[:, bs], in0=lt_acc[:, bs], scalar1=bias_sb
            )

        # transpose logits^T [C, bags] -> [bags, C] and store
        for t in range(bags // 128):
            pt = psum.tile([128, C], F32, tag="pt", bufs=1)
            nc.tensor.transpose(
                pt, lt_acc[:, t * 128 : (t + 1) * 128], ident
            )
            nc.vector.tensor_copy(logits_sb[:, t, :], pt)
        nc.sync.dma_start(
            out=out.rearrange("(t p) c -> p t c", p=128), in_=logits_sb
        )
    return nc


_NC_CACHE: dict = {}


def _get_nc(rows: int) -> bass.Bass:
    if rows not in _NC_CACHE:
        nc = build_nc(rows)
        nc.finalize()
        _NC_CACHE[rows] = nc
    return _NC_CACHE[rows]


def _numpy_fallback(x, rel_weight, bias, input_scope, query):
    """Pure-numpy replication of the reference for non-uniform bag layouts."""
    n = x.shape[0]
    num_bags = input_scope.shape[0] - 1
    seg = np.searchsorted(input_scope[1:], np.arange(n), side="right")
    att = np.einsum("nd,nd->n", x, rel_weight[query]).astype(np.float32)
    valid = seg < num_bags
    segv = seg[valid]
    attv = att[valid]
    m = np.full(num_bags, -np.inf, dtype=np.float32)
    np.maximum.at(m, segv, attv)
    e = np.zeros(n, dtype=np.float32)
    e[valid] = np.exp(attv - m[segv])
    z = np.zeros(num_bags, dtype=np.float32)
    np.add.at(z, segv, e[valid])
    w = np.zeros(n, dtype=np.float32)
    nz = z[segv] != 0
    w_valid = np.zeros(segv.shape[0], dtype=np.float32)
    w_valid[nz] = e[valid][nz] / z[segv][nz]
    w[valid] = w_valid
    repre = np.zeros((num_bags, x.shape[1]), dtype=np.float32)
    np.add.at(repre, segv, (x[valid] * w[valid][:, None]).astype(np.float32))
    return repre @ rel_weight.T + bias


def _split_f16(a):
    hi = a.astype(np.float16)
    lo = (a - hi.astype(np.float32)).astype(np.float16)
    return hi, lo


def _prepare_in_maps(x, rel_weight, bias, query):
    rh, rl = _split_f16(rel_weight)  # [C, D] each
    relt2 = np.zeros((D, 2, 64), dtype=np.float16)
    relt2[:, 0, :C] = rh.T
    relt2[:, 1, :C] = rl.T
    sident = np.zeros((128, C), dtype=np.float32)
    sident[np.arange(C), np.arange(C)] = 1.0
    sident[64 + np.arange(C), np.arange(C)] = 1.0
    identm = np.eye(C, dtype=np.float32)
    biast = np.ascontiguousarray(bias.reshape(C, 1)).astype(np.float32)
    q = query.astype(np.int64)
    in_maps = []
    for c in range(N_CORES):
        lo_r, hi_r = c * ROWS, (c + 1) * ROWS
        xh, xl = _split_f16(x[lo_r:hi_r])
        xt2 = np.empty((D, 2, ROWS), dtype=np.float16)
        xt2[:, 0, :] = xh.T
        xt2[:, 1, :] = xl.T
        oh = np.zeros((128, ROWS), dtype=np.float16)
        qc = q[lo_r:hi_r]
        ar = np.arange(ROWS)
        oh[qc, ar] = 1.0
        oh[64 + qc, ar] = 1.0
        in_maps.append(
            {"xt2": xt2, "oht": oh, "relt2": relt2, "sident": sident,
             "identm": identm, "biast": biast}
        )
    return in_maps


def run_device(x, rel_weight, bias, query, trace=False, **kwargs):
    nc = _get_nc(ROWS)
    in_maps = _prepare_in_maps(x, rel_weight, bias, query)
    res = run_bass_kernel_spmd(
        nc, in_maps, core_ids=list(range(N_CORES)), trace=trace, **kwargs
    )
    outs = [np.asarray(r["out"]) for r in res.results]
    return np.concatenate(outs, axis=0), res


def kernel(x, rel_weight, bias, input_scope, query):
    x = np.asarray(x, dtype=np.float32)
    rel_weight = np.asarray(rel_weight, dtype=np.float32)
    bias = np.asarray(bias, dtype=np.float32)
    input_scope = np.asarray(input_scope)
    query = np.asarray(query)

    expected_scope = np.arange(B + 1, dtype=np.int64) * (N // B)
    if (
        x.shape == (N, D)
        and rel_weight.shape == (C, D)
        and input_scope.shape == (B + 1,)
        and np.array_equal(input_scope.astype(np.int64), expected_scope)
    ):
        out, _ = run_device(x, rel_weight, bias, query)
        return out
    return _numpy_fallback(x, rel_weight, bias, input_scope, query)


# revision 31
# speedup vs baseline: 1.4737x; 1.4737x over previous
"""Trainium2 Bass kernel for bag-level attention (ragged_sequence).

Math (per bag b over its 16 sentences i):
    att_i  = <x_i, rel[q_i]>
    w      = softmax(att) within bag
    logits = (sum_i w_i x_i) @ rel.T + bias

Key identity: logits[b] = sum_i w_i S[i,:] + bias with S = x @ rel.T, so x is
read from HBM exactly once.

Precision: x and rel are split on the host into fp16 hi + fp16 lo
(x = hi + lo, 22-bit combined mantissa). The four partial products
hi*hi + hi*lo + lo*hi + lo*lo are accumulated in fp32 PSUM, reproducing fp32
accuracy (~1e-6 rel) while running the TensorE at full fp16 rate (fp32
matmuls run at quarter rate and do not warm the HAM clock gate).

Device layout (per core, rows = N/8 sentences):
    S.T split over two partition blocks of PSUM st[128, ch]:
      rows 0:64   = relT_hi(64-col zero-padded).T @ xT_{hi,lo}   (tile_position (0,0))
      rows 64:128 = relT_lo(padded).T @ xT_{hi,lo}               (tile_position (0,64))
    The two col-tiles share each moving stream (concurrent sub-array execution).
    att  = partition_all_reduce(st * onehot2)        (GpSimd; onehot2 has the
           one-hot replicated in both blocks, built on host)
    e    = exp(att)                                  (ScalarE)
    ebs  = partition_broadcast(e)                    (GpSimd)
    lu[128, bags] = windowed reduce_16(st * ebs)     (VectorE)
    logitsU.T[53, bags] = stacked_identity.T @ lu    (recombines hi+lo blocks)
    * 1/z, + bias, final PE transpose to [bags, 53].
"""

import os
from contextlib import ExitStack

import numpy as np

import concourse.bass as bass
import concourse.tile as tile
from concourse import bacc, library_config, mybir
from concourse.bass_utils import run_bass_kernel_spmd

# Problem constants (hardcoded per spec nn_Attention_85478439125349)
N = 262144
B = 16384
D = 768
C = 53
BAG = 16
N_CORES = 8
ROWS = N // N_CORES          # 32768 sentences per core
BAGS = B // N_CORES          # 2048 bags per core
KCH = D // 128               # 6 contraction chunks
F32 = mybir.dt.float32
F16 = mybir.dt.float16


def build_nc(rows: int, sc: int = 2048, ch: int = 512) -> bass.Bass:
    """Build the per-core Bass program for `rows` sentences (bags of BAG)."""
    assert rows % sc == 0 and sc % ch == 0 and ch % BAG == 0
    bags = rows // BAG
    n_sc = rows // sc          # superchunks (DMA granularity)
    n_ch = sc // ch            # compute chunks per superchunk
    chb = ch // BAG            # bags per compute chunk (32)
    scb = sc // BAG            # bags per superchunk (128)

    nc = bacc.Bacc()
    # hi/lo interleaved: xt2[d, 0, r] = fp16 hi, xt2[d, 1, r] = fp16 lo
    xt2 = nc.declare_dram_parameter("xt2", [D, 2, rows], F16, isOutput=False)
    # one-hot mask replicated into both partition blocks: [128, rows]
    oht = nc.declare_dram_parameter("oht", [128, rows], F16, isOutput=False)
    # relT hi/lo, each zero-padded to 64 output columns: [D, 2, 64]
    relt2 = nc.declare_dram_parameter("relt2", [D, 2, 64], F16, isOutput=False)
    # stacked identity [128, C]: row k -> col m if k==m or k==64+m
    sident = nc.declare_dram_parameter("sident", [128, C], F32, isOutput=False)
    identm = nc.declare_dram_parameter("identm", [C, C], F32, isOutput=False)
    biast = nc.declare_dram_parameter("biast", [C, 1], F32, isOutput=False)
    out = nc.declare_dram_parameter("out", [bags, C], F32, isOutput=True)

    xt_v = xt2.rearrange("(k p) h r -> k p h r", p=128)      # [KCH, 128, 2, rows]
    relt_v = relt2.rearrange("(k p) h c -> k p h c", p=128)  # [KCH, 128, 2, 64]

    with tile.TileContext(nc) as tc, ExitStack() as ctx:
        consts = ctx.enter_context(tc.tile_pool(name="consts", bufs=1))
        xpool = ctx.enter_context(tc.tile_pool(name="xpool", bufs=2))
        ohpool = ctx.enter_context(tc.tile_pool(name="ohpool", bufs=2))
        work = ctx.enter_context(tc.tile_pool(name="work", bufs=3))
        psum = ctx.enter_context(tc.tile_pool(name="psum", bufs=2, space="PSUM"))

        # --- constants ---
        relt_sb = consts.tile([128, KCH, 2, 64], F16)
        nc.sync.dma_start(out=relt_sb, in_=relt_v.transpose([1, 0, 2, 3]))
        sident_sb = consts.tile([128, C], F32)
        nc.sync.dma_start(out=sident_sb, in_=sident[:, :])
        bias_sb = consts.tile([C, 1], F32)
        nc.sync.dma_start(out=bias_sb, in_=biast[:, :])
        ident = consts.tile([C, C], F32)
        nc.sync.dma_start(out=ident, in_=identm[:, :])
        zeros_sb = consts.tile([64, 512], F32)
        nc.vector.memset(zeros_sb, 0.0)
        ones128 = consts.tile([128, 1], F32)
        nc.vector.memset(ones128, 1.0)
        nc.gpsimd.load_library(library_config.attn)
        # accumulator for logits^T [C, bags] and staging for transposed output
        lt_acc = consts.tile([C, bags], F32)
        logits_sb = consts.tile([128, bags // 128, C], F32)

        for isc in range(n_sc):
            x_sb = xpool.tile([128, KCH, 2, sc], F16)
            for h in range(2):
                nc.sync.dma_start(
                    out=x_sb[:, :, h, :],
                    in_=xt_v[:, :, h, isc * sc : (isc + 1) * sc].transpose([1, 0, 2]),
                )
            oh_sb = ohpool.tile([128, sc], F16)
            nc.sync.dma_start(out=oh_sb, in_=oht[:, isc * sc : (isc + 1) * sc])

            for ic in range(n_ch):
                cs = slice(ic * ch, (ic + 1) * ch)
                st = psum.tile([128, ch], F32, tag="st", bufs=4)
                # Zero the lo block: its matmuls use start=False (a second
                # bank-wide has_written clear would wipe the hi block), so on
                # sim/stale PSUM the first accumulate needs zeroed ground.
                nc.scalar.copy(st[64:128, :], zeros_sb[:, :ch])
                for k in range(KCH):
                    # hi block: r_hi against both x streams
                    for h in range(2):
                        nc.tensor.matmul(
                            st[0:64, :],
                            lhsT=relt_sb[:, k, 0, :],
                            rhs=x_sb[:, k, h, cs],
                            start=(k == 0 and h == 0),
                            stop=(k == KCH - 1 and h == 1),
                            tile_position=(0, 0),
                        )
                    # lo block: r_lo against x_hi only (lo*lo term ~2^-22, dropped)
                    nc.tensor.matmul(
                        st[64:128, :],
                        lhsT=relt_sb[:, k, 1, :],
                        rhs=x_sb[:, k, 0, cs],
                        start=False,
                        stop=False,
                        skip_group_check=True,
                        tile_position=(0, 64),
                    )
                # att extraction: sm = st * onehot2 ; att = column sums via
                # fp32 ones-matmul (partition reduction on TensorE)
                sm = work.tile([128, ch], F32, tag="sm")
                nc.vector.tensor_mul(sm, st, oh_sb[:, cs])
                att = psum.tile([1, ch], F32, tag="att", bufs=1)
                nc.tensor.matmul(att, lhsT=ones128, rhs=sm)
                e = work.tile([1, ch], F32, tag="e")
                nc.scalar.activation(
                    e, att, mybir.ActivationFunctionType.Exp
                )
                # broadcast e across partitions, weight S^T, bag-reduce
                ebs = work.tile([128, ch], F32, tag="ebs")
                nc.gpsimd.partition_broadcast(ebs, e, channels=128)
                w = work.tile([128, ch], F32, tag="w")
                nc.vector.tensor_mul(w, st, ebs)
                lu = work.tile([128, chb], F32, tag="lu")
                nc.vector.reduce_sum(
                    lu, w.rearrange("p (b j) -> p b j", j=BAG), axis=mybir.AxisListType.X
                )
                # recombine hi+lo partition blocks: [53, chb]
                lc = psum.tile([C, chb], F32, tag="lc")
                nc.tensor.matmul(lc, lhsT=sident_sb, rhs=lu)
                # z per bag from the broadcast copy; normalize
                zb = work.tile([C, chb], F32, tag="zb")
                nc.vector.reduce_sum(
                    zb,
                    ebs[0:C, :].rearrange("p (b j) -> p b j", j=BAG),
                    axis=mybir.AxisListType.X,
                )
                rzb = work.tile([C, chb], F32, tag="rzb")
                nc.vector.reciprocal(rzb, zb)
                ob = isc * scb + ic * chb
                nc.vector.tensor_mul(lt_acc[:, ob : ob + chb], lc, rzb)
            # bias for this superchunk's bag block
            bs = slice(isc * scb, (isc + 1) * scb)
            nc.vector.tensor_scalar_add(
                out=lt_acc[:, bs], in0=lt_acc[:, bs], scalar1=bias_sb
            )

        # transpose logits^T [C, bags] -> [bags, C] and store
        for t in range(bags // 128):
            pt = psum.tile([128, C], F32, tag="pt", bufs=1)
            nc.tensor.transpose(
                pt, lt_acc[:, t * 128 : (t + 1) * 128], ident
            )
            nc.vector.tensor_copy(logits_sb[:, t, :], pt)
        nc.sync.dma_start(
            out=out.rearrange("(t p) c -> p t c", p=128), in_=logits_sb
        )
    return nc


_NC_CACHE: dict = {}


def _get_nc(rows: int) -> bass.Bass:
    if rows not in _NC_CACHE:
        nc = build_nc(rows)
        nc.finalize()
        _NC_CACHE[rows] = nc
    return _NC_CACHE[rows]


def _numpy_fallback(x, rel_weight, bias, input_scope, query):
    """Pure-numpy replication of the reference for non-uniform bag layouts."""
    n = x.shape[0]
    num_bags = input_scope.shape[0] - 1
    seg = np.searchsorted(input_scope[1:], np.arange(n), side="right")
    att = np.einsum("nd,nd->n", x, rel_weight[query]).astype(np.float32)
    valid = seg < num_bags
    segv = seg[valid]
    attv = att[valid]
    m = np.full(num_bags, -np.inf, dtype=np.float32)
    np.maximum.at(m, segv, attv)
    e = np.zeros(n, dtype=np.float32)
    e[valid] = np.exp(attv - m[segv])
    z = np.zeros(num_bags, dtype=np.float32)
    np.add.at(z, segv, e[valid])
    w = np.zeros(n, dtype=np.float32)
    nz = z[segv] != 0
    w_valid = np.zeros(segv.shape[0], dtype=np.float32)
    w_valid[nz] = e[valid][nz] / z[segv][nz]
    w[valid] = w_valid
    repre = np.zeros((num_bags, x.shape[1]), dtype=np.float32)
    np.add.at(repre, segv, (x[valid] * w[valid][:, None]).astype(np.float32))
    return repre @ rel_weight.T + bias


def _split_f16(a):
    hi = a.astype(np.float16)
    lo = (a - hi.astype(np.float32)).astype(np.float16)
    return hi, lo


def _prepare_in_maps(x, rel_weight, bias, query):
    rh, rl = _split_f16(rel_weight)  # [C, D] each
    relt2 = np.zeros((D, 2, 64), dtype=np.float16)
    relt2[:, 0, :C] = rh.T
    relt2[:, 1, :C] = rl.T
    sident = np.zeros((128, C), dtype=np.float32)
    sident[np.arange(C), np.arange(C)] = 1.0
    sident[64 + np.arange(C), np.arange(C)] = 1.0
    identm = np.eye(C, dtype=np.float32)
    biast = np.ascontiguousarray(bias.reshape(C, 1)).astype(np.float32)
    q = query.astype(np.int64)
    in_maps = []
    for c in range(N_CORES):
        lo_r, hi_r = c * ROWS, (c + 1) * ROWS
        xh, xl = _split_f16(x[lo_r:hi_r])
        xt2 = np.empty((D, 2, ROWS), dtype=np.float16)
        xt2[:, 0, :] = xh.T
        xt2[:, 1, :] = xl.T
        oh = np.zeros((128, ROWS), dtype=np.float16)
        qc = q[lo_r:hi_r]
        ar = np.arange(ROWS)
        oh[qc, ar] = 1.0
        oh[64 + qc, ar] = 1.0
        in_maps.append(
            {"xt2": xt2, "oht": oh, "relt2": relt2, "sident": sident,
             "identm": identm, "biast": biast}
        )
    return in_maps


def run_device(x, rel_weight, bias, query, trace=False, **kwargs):
    nc = _get_nc(ROWS)
    in_maps = _prepare_in_maps(x, rel_weight, bias, query)
    res = run_bass_kernel_spmd(
        nc, in_maps, core_ids=list(range(N_CORES)), trace=trace, **kwargs
    )
    outs = [np.asarray(r["out"]) for r in res.results]
    return np.concatenate(outs, axis=0), res


def kernel(x, rel_weight, bias, input_scope, query):
    x = np.asarray(x, dtype=np.float32)
    rel_weight = np.asarray(rel_weight, dtype=np.float32)
    bias = np.asarray(bias, dtype=np.float32)
    input_scope = np.asarray(input_scope)
    query = np.asarray(query)

    expected_scope = np.arange(B + 1, dtype=np.int64) * (N // B)
    if (
        x.shape == (N, D)
        and rel_weight.shape == (C, D)
        and input_scope.shape == (B + 1,)
        and np.array_equal(input_scope.astype(np.int64), expected_scope)
    ):
        out, _ = run_device(x, rel_weight, bias, query)
        return out
    return _numpy_fallback(x, rel_weight, bias, input_scope, query)
